# revision 1
# baseline (speedup 1.0000x reference)
"""Multi-head causal attention (B=1, T=4096, D=768, H=12) on 8 trn2 cores.

Sharding: 16 uniform head-slots (2 per core), 12 real heads + 4 dummy
(zero-weight) slots.  Every core runs the IDENTICAL program (SPMD); cores
differ only in the weight data they receive.  Each core computes, for its
two head-slots, the full causal attention over all 4096 tokens plus that
slot-pair's partial output projection (out.T = wo_slice.T @ headout).  The
host sums the 8 partial [768, 4096] outputs, transposes, and adds the
output bias.

On-device layout (per core):
  xT   [768, 4096]  bf16   x transposed (host supplies)
  QT/KT[128, 4096]  bf16   partitions 0:64 head A dims, 64:128 head B dims
  VT   [128, 4096]  bf16   same, then PE-transposed to V [tok, dims]
  scoresT chunks [128 keys, 256 queries] in PSUM, exp'd on ACT -> PT bf16
  AV + denominators accumulated in PSUM via (V | ones) packed matmuls
"""

import math
import os
import numpy as np
import ml_dtypes
from contextlib import ExitStack

import concourse.bass as bass
import concourse.bacc as bacc
import concourse.mybir as mybir
import concourse.tile as tile
from concourse.bass_utils import run_bass_kernel_spmd

BF16 = mybir.dt.bfloat16
F32 = mybir.dt.float32
AF = mybir.ActivationFunctionType

T = 4096
D_MODEL = 768
HEAD_DIM = 64
N_HEADS = 12
N_CORES = 8
QT = 512                  # query tile width (one full PSUM bank per chunk)
KC = 128                  # key chunk (psum partition dim)
GRP = 3                   # score chunk-jobs per exp group -> ACT free dim 1536
NQT = T // QT             # 8 query tiles
CCH = D_MODEL // 128      # 6 contraction chunks
TOKT = 512                # token tile for projections
NTOKT = T // TOKT

_PROGRAM_CACHE = {}


def build_program(n_qtiles=NQT, skip_attn=False, attn_stage=7):
    nc = bacc.Bacc(None)

    xT_d = nc.declare_dram_parameter("xT", [D_MODEL, T], BF16, isOutput=False)
    w_d = nc.declare_dram_parameter("wqkv", [3, D_MODEL, 128], BF16, isOutput=False)
    b_d = nc.declare_dram_parameter("bqkv", [128, 3], F32, isOutput=False)
    wo_d = nc.declare_dram_parameter("wo2", [128, D_MODEL], BF16, isOutput=False)
    mk_d = nc.declare_dram_parameter("masks", [4, 128, QT], BF16, isOutput=False)
    id_d = nc.declare_dram_parameter("ident", [128, 128], BF16, isOutput=False)
    outT_d = nc.declare_dram_parameter("outT", [D_MODEL, T], F32, isOutput=True)

    with tile.TileContext(nc) as tc, ExitStack() as ctx:
        consts = ctx.enter_context(tc.tile_pool(name="consts", bufs=1))
        big = ctx.enter_context(tc.tile_pool(name="big", bufs=1))
        ptp = ctx.enter_context(tc.tile_pool(name="ptp", bufs=3))
        rp = ctx.enter_context(tc.tile_pool(name="rp", bufs=2))
        osb = ctx.enter_context(tc.tile_pool(name="osb", bufs=3))
        # PSUM: score/proj/outproj pool 3 banks x2, av 2 banks x1 = 8 banks
        sp = ctx.enter_context(tc.tile_pool(name="sp", bufs=2, space="PSUM"))
        avp = ctx.enter_context(tc.tile_pool(name="avp", bufs=1, space="PSUM"))
        dramp = ctx.enter_context(tc.tile_pool(name="dramp", bufs=2, space="DRAM"))

        # ---- constants / inputs to SBUF ----
        xT_sb = []
        for j in range(CCH):
            t = big.tile([128, T], BF16, tag=f"xT{j}")
            nc.sync.dma_start(out=t[:], in_=xT_d[j * 128:(j + 1) * 128, :])
            xT_sb.append(t)
        w_sb = consts.tile([128, 3 * CCH * 128], BF16, tag="w")
        for s in range(3):
            for j in range(CCH):
                nc.sync.dma_start(
                    out=w_sb[:, (s * CCH + j) * 128:(s * CCH + j + 1) * 128],
                    in_=w_d[s, j * 128:(j + 1) * 128, :],
                )
        b_sb = consts.tile([128, 3], F32, tag="b")
        nc.sync.dma_start(out=b_sb[:], in_=b_d[:, :])
        wo_sb = consts.tile([128, D_MODEL], BF16, tag="wo")
        nc.sync.dma_start(out=wo_sb[:], in_=wo_d[:, :])
        mask_sb = consts.tile([128, 4 * QT], BF16, tag="mask")
        for p in range(4):
            nc.sync.dma_start(out=mask_sb[:, p * QT:(p + 1) * QT], in_=mk_d[p, :, :])
        id_sb = consts.tile([128, 128], BF16, tag="id")
        nc.sync.dma_start(out=id_sb[:], in_=id_d[:, :])

        # ---- projections: QT/KT/VT [128(A|B dims), T] ----
        qkv_sb = []
        for s in range(3):
            t = big.tile([128, T], BF16, tag=f"qkv{s}")
            qkv_sb.append(t)
        for s in range(3):
            for tt in range(NTOKT):
                # head-A accumulation group in bank 0, head-B in bank 1
                pp = sp.tile([128, 2 * TOKT], F32, tag="sc")
                for j in range(CCH):
                    base = (s * CCH + j) * 128
                    rhs = xT_sb[j][:, tt * TOKT:(tt + 1) * TOKT]
                    nc.tensor.matmul(
                        pp[0:64, 0:TOKT], w_sb[:, base:base + 64], rhs,
                        start=(j == 0), stop=(j == CCH - 1), tile_position=(0, 0),
                    )
                    nc.tensor.matmul(
                        pp[64:128, TOKT:2 * TOKT], w_sb[:, base + 64:base + 128], rhs,
                        start=(j == 0), stop=(j == CCH - 1), tile_position=(0, 64),
                    )
                nc.vector.tensor_scalar_add(
                    qkv_sb[s][0:64, tt * TOKT:(tt + 1) * TOKT],
                    pp[0:64, 0:TOKT], b_sb[0:64, s:s + 1],
                )
                nc.vector.tensor_scalar_add(
                    qkv_sb[s][64:128, tt * TOKT:(tt + 1) * TOKT],
                    pp[64:128, TOKT:2 * TOKT], b_sb[64:128, s:s + 1],
                )
        QT_sb, KT_sb, VT_sb = qkv_sb

        # ---- V2 per 128-token key tile, stride 208 cols:
        #   [0:64]=V_A  [64:65]=1  [97:98]=1  [129:193]=V_B  (rest 0)
        # lhsT A = cols 0:65  -> psum rows 0:64 AV_A, row 64 denom_A
        # lhsT B = cols 65:193 -> psum row 32 denom_B, rows 64:128 AV_B
        #                         (single accumulation group per bank)
        VST = 208
        V_sb = big.tile([128, (T // 128) * VST], BF16, tag="V")
        nc.vector.memset(V_sb[:], 0.0)
        v3 = V_sb[:].rearrange("p (t c) -> p t c", c=VST)
        nc.vector.memset(v3[:, :, 64:65], 1.0)
        nc.vector.memset(v3[:, :, 97:98], 1.0)
        for tt4 in range(T // 128):
            tp = sp.tile([128, 128], BF16, tag="sc")
            nc.tensor.transpose(tp[:], VT_sb[:, tt4 * 128:(tt4 + 1) * 128], id_sb[:])
            nc.vector.tensor_copy(V_sb[:, tt4 * VST:tt4 * VST + 64], tp[:, 0:64])
            nc.vector.tensor_copy(
                V_sb[:, tt4 * VST + 129:tt4 * VST + 193], tp[:, 64:128])

        # ---- attention + out-projection ----
        ho_all = big.tile([128, T], BF16, tag="ho")
        if skip_attn:
            nc.vector.memset(ho_all[:], 0.0)
        for qi in range(n_qtiles):
            qs = qi * QT
            if skip_attn:
                for dch in range(CCH):
                    op = sp.tile([128, QT], F32, tag="sc")
                    nc.tensor.matmul(
                        op[:], wo_sb[:, dch * 128:(dch + 1) * 128],
                        ho_all[:, qs:qs + QT], start=True, stop=True,
                    )
                    ot = osb.tile([128, QT], F32, tag="ot")
                    nc.vector.tensor_copy(ot[:], op[:])
                    nc.sync.dma_start(
                        out=outT_d[dch * 128:(dch + 1) * 128, qs:qs + QT], in_=ot[:])
                continue
            # av bank 0: head-A group (AV rows 0:64, denom row 64)
            # av bank 1: head-B group (denom row 32, AV rows 64:128)
            av = None
            if attn_stage >= 4:
                av = avp.tile([128, 2 * QT], F32, tag="av")
            nsteps = 4 * (qi + 1)
            # one chunk job = [128 keys x QT queries] scores for one head
            # = exactly one PSUM bank; jobs grouped GRP at a time for exp
            jobs = [(kc, h) for kc in range(nsteps) for h in (0, 1)]
            for g in range(0, len(jobs), GRP):
                grp = jobs[g:g + GRP]
                width = len(grp) * QT
                sc = sp.tile([128, GRP * QT], F32, tag="sc")
                for ji, (kc, h) in enumerate(grp):
                    nc.tensor.matmul(
                        sc[:, ji * QT:(ji + 1) * QT],
                        KT_sb[64 * h:64 * h + 64, kc * KC:(kc + 1) * KC],
                        QT_sb[64 * h:64 * h + 64, qs:qs + QT],
                        start=True, stop=True, tile_position=(64 * h, 0),
                    )
                if attn_stage < 2:
                    continue
                pt = ptp.tile([128, GRP * QT], BF16, tag="pt")
                nc.scalar.activation(
                    pt[:, :width], sc[:, :width], AF.Exp,
                    scale=1.0 / math.sqrt(HEAD_DIM),
                )
                for ji, (kc, h) in enumerate(grp):
                    ptj = pt[:, ji * QT:(ji + 1) * QT]
                    if attn_stage >= 3 and kc >= 4 * qi:  # diagonal straddle
                        pat = kc - 4 * qi
                        m = mask_sb[:, pat * QT:(pat + 1) * QT]
                        nc.vector.tensor_mul(ptj, ptj, m)
                    if attn_stage < 4:
                        continue
                    st = kc == 0
                    sp_ = kc == nsteps - 1
                    vbase = kc * 208
                    if h == 0:
                        nc.tensor.matmul(
                            av[0:65, 0:QT], V_sb[:, vbase:vbase + 65], ptj,
                            start=st, stop=sp_, tile_position=(0, 0),
                        )
                    else:
                        nc.tensor.matmul(
                            av[0:128, QT:2 * QT], V_sb[:, vbase + 65:vbase + 193], ptj,
                            start=st, stop=sp_, tile_position=(0, 0),
                        )
            if attn_stage < 5:
                continue
            # normalize: recip of denominators, partition-broadcast via DRAM
            r = rp.tile([128, 2 * QT], F32, tag="r")
            nc.vector.reciprocal(r[64:65, 0:QT], av[64:65, 0:QT])
            nc.vector.reciprocal(r[32:33, QT:2 * QT], av[32:33, QT:2 * QT])
            # partition-broadcast via DRAM bounce (stride-0 partition reads
            # are only legal from DRAM)
            if attn_stage < 6:
                continue
            rd = dramp.tile([1, 2 * QT], F32, tag="rd")
            nc.sync.dma_start(out=rd[0:1, 0:QT], in_=r[64:65, 0:QT])
            nc.sync.dma_start(out=rd[0:1, QT:2 * QT], in_=r[32:33, QT:2 * QT])
            rbc = rp.tile([128, QT], F32, tag="rbc")
            rdA = rd[0:1, 0:QT]
            rdB = rd[0:1, QT:2 * QT]
            nc.gpsimd.dma_start(
                out=rbc[0:64, :],
                in_=bass.AP(tensor=rdA.tensor, offset=rdA.offset,
                            ap=[[0, 64]] + list(rdA.ap[1:])))
            nc.gpsimd.dma_start(
                out=rbc[64:128, :],
                in_=bass.AP(tensor=rdB.tensor, offset=rdB.offset,
                            ap=[[0, 64]] + list(rdB.ap[1:])))
            if attn_stage < 7:
                continue
            nc.vector.tensor_mul(
                ho_all[0:64, qs:qs + QT], av[0:64, 0:QT], rbc[0:64, :])
            nc.vector.tensor_mul(
                ho_all[64:128, qs:qs + QT], av[64:128, QT:2 * QT], rbc[64:128, :])
            # out projection for this query tile: outT[dout, q]
            for dch in range(CCH):
                op = sp.tile([128, QT], F32, tag="sc")
                nc.tensor.matmul(
                    op[:], wo_sb[:, dch * 128:(dch + 1) * 128],
                    ho_all[:, qs:qs + QT], start=True, stop=True,
                )
                ot = osb.tile([128, QT], F32, tag="ot")
                nc.vector.tensor_copy(ot[:], op[:])
                nc.sync.dma_start(
                    out=outT_d[dch * 128:(dch + 1) * 128, qs:qs + QT], in_=ot[:],
                )
    nc.finalize()
    return nc


def _host_inputs(x, wq, bq, wk, bk, wv, bv, wo):
    """Per-core input maps. Slot A of core c = head c; slot B = head 8+c
    (cores 0-3) or a dummy zero head (cores 4-7)."""
    bf16 = ml_dtypes.bfloat16
    xT = np.ascontiguousarray(x[0].T).astype(bf16)
    masks = np.zeros((4, 128, QT), np.float32)
    dk = np.arange(128)[:, None]
    dq = np.arange(QT)[None, :]
    for p in range(4):
        masks[p] = (dk + 128 * p <= dq)
    masks = masks.astype(bf16)
    ident = np.eye(128, dtype=np.float32).astype(bf16)

    in_maps = []
    for c in range(N_CORES):
        hA = c
        hB = 8 + c if c < 4 else None
        w = np.zeros((3, D_MODEL, 128), np.float32)
        b = np.zeros((128, 3), np.float32)
        wo2 = np.zeros((128, D_MODEL), np.float32)
        for s, (W, B) in enumerate(((wq, bq), (wk, bk), (wv, bv))):
            w[s, :, 0:64] = W[hA]
            b[0:64, s] = B[hA]
            if hB is not None:
                w[s, :, 64:128] = W[hB]
                b[64:128, s] = B[hB]
        wo2[0:64, :] = wo[hA * 64:(hA + 1) * 64, :]
        if hB is not None:
            wo2[64:128, :] = wo[hB * 64:(hB + 1) * 64, :]
        in_maps.append({
            "xT": xT,
            "wqkv": w.astype(bf16),
            "bqkv": b.astype(np.float32),
            "wo2": wo2.astype(bf16),
            "masks": masks,
            "ident": ident,
        })
    return in_maps


def kernel(_trace=False, _tmpdir=None, **inputs):
    x = np.asarray(inputs["x"], np.float32)
    args = (x,
            np.asarray(inputs["wq"], np.float32), np.asarray(inputs["bq"], np.float32),
            np.asarray(inputs["wk"], np.float32), np.asarray(inputs["bk"], np.float32),
            np.asarray(inputs["wv"], np.float32), np.asarray(inputs["bv"], np.float32),
            np.asarray(inputs["wo"], np.float32))
    bo = np.asarray(inputs["bo"], np.float32)

    if "nc" not in _PROGRAM_CACHE:
        _PROGRAM_CACHE["nc"] = build_program()
    nc = _PROGRAM_CACHE["nc"]

    in_maps = _host_inputs(*args)
    res = run_bass_kernel_spmd(
        nc, in_maps, list(range(N_CORES)), trace=_trace, tmpdir=_tmpdir,
    )
    acc = np.zeros((D_MODEL, T), np.float32)
    for c in range(N_CORES):
        acc += res.results[c]["outT"]
    out = acc.T + bo[None, :]
    if _trace:
        return out[None].astype(np.float32), res
    return out[None].astype(np.float32)



# revision 4
# speedup vs baseline: 1.0545x; 1.0545x over previous
"""Multi-head causal attention (B=1, T=4096, D=768, H=12) on 8 trn2 cores.

Sharding: 16 uniform head-slots (2 per core), 12 real heads + 4 dummy
(zero-weight) slots.  Every core runs the IDENTICAL program (SPMD); cores
differ only in the weight data they receive.  Each core computes, for its
two head-slots, the full causal attention over all 4096 tokens plus that
slot-pair's partial output projection.  The host sums the 8 partial
[768, 4096] bf16 outputs, transposes, and adds the output bias.

v2 layout (per core):
  xT    [768, 4096] bf16   x transposed (host supplies), DMA'd per tok-tile
  Q8/K8 [128, 2T]   fp8e4  partitions 0:64 slot A dims, 64:128 slot B;
                           cols T:2T are zeros (DoubleRow 2nd k-subtile pad)
  VT    [128, 4096] bf16   V pre-transpose, then PE-transposed into
  V2    [128, 32*256] bf16 per key chunk c: [V_A | ones x128 | V_B]
                           -> AV matmul lhsT [V_A|ones64] / [ones64|V_B]
                           gives AV rows + 64x-replicated denominators
  scores: fp8 DoubleRow matmuls -> PSUM f32 -> ACT exp -> pt bf16
  normalize: DVE reciprocal of the replicated denom block, DRAM-bounce
  partition broadcast, division fused into the PSUM->SBUF copy, merged
  (k=128) out-projection, bf16 output.
"""

import math
import numpy as np
import ml_dtypes
from contextlib import ExitStack

import concourse.bass as bass
import concourse.bacc as bacc
import concourse.mybir as mybir
import concourse.tile as tile
from concourse.bass_utils import run_bass_kernel_spmd

BF16 = mybir.dt.bfloat16
FP8 = mybir.dt.float8e4
F32 = mybir.dt.float32
AF = mybir.ActivationFunctionType
PM = mybir.MatmulPerfMode

T = 4096
D_MODEL = 768
HEAD_DIM = 64
N_HEADS = 12
N_CORES = 8
QT = 512                  # query tile width (one full PSUM bank per chunk)
KC = 128                  # key chunk (psum partition dim)
GRP = 3                   # score chunk-jobs per exp group -> ACT free dim 1536
NQT = T // QT             # 8 query tiles
CCH = D_MODEL // 128      # 6 contraction chunks
TOKT = 512                # token tile for projections
NTOKT = T // TOKT
VST = 256                 # V2 stride per 128-key chunk
SCORES_FP8 = True

_PROGRAM_CACHE = {}


def build_program():
    nc = bacc.Bacc(None)

    xT_d = nc.declare_dram_parameter("xT", [D_MODEL, T], BF16, isOutput=False)
    w_d = nc.declare_dram_parameter("wqkv", [3, D_MODEL, 128], BF16, isOutput=False)
    b_d = nc.declare_dram_parameter("bqkv", [128, 3], F32, isOutput=False)
    wo_d = nc.declare_dram_parameter("wo2", [128, D_MODEL], BF16, isOutput=False)
    mk_d = nc.declare_dram_parameter("masks", [4, 128, QT], BF16, isOutput=False)
    id_d = nc.declare_dram_parameter("ident", [128, 128], BF16, isOutput=False)
    outT_d = nc.declare_dram_parameter("outT", [D_MODEL, T], BF16, isOutput=True)

    qk_dt = FP8 if SCORES_FP8 else BF16

    with tile.TileContext(nc) as tc, ExitStack() as ctx:
        consts = ctx.enter_context(tc.tile_pool(name="consts", bufs=1))
        big = ctx.enter_context(tc.tile_pool(name="big", bufs=1))
        ptp = ctx.enter_context(tc.tile_pool(name="ptp", bufs=3))
        rp = ctx.enter_context(tc.tile_pool(name="rp", bufs=2))
        hvp = ctx.enter_context(tc.tile_pool(name="hvp", bufs=2))
        osb = ctx.enter_context(tc.tile_pool(name="osb", bufs=3))
        # PSUM: score/proj/outproj pool 3 banks x2, av 2 banks x1 = 8 banks
        sp = ctx.enter_context(tc.tile_pool(name="sp", bufs=2, space="PSUM"))
        avp = ctx.enter_context(tc.tile_pool(name="avp", bufs=1, space="PSUM"))
        dramp = ctx.enter_context(tc.tile_pool(name="dramp", bufs=2, space="DRAM"))

        # ---- constants to SBUF ----
        w_sb = consts.tile([128, 3 * CCH * 128], BF16, tag="w")
        for s in range(3):
            for j in range(CCH):
                nc.sync.dma_start(
                    out=w_sb[:, (s * CCH + j) * 128:(s * CCH + j + 1) * 128],
                    in_=w_d[s, j * 128:(j + 1) * 128, :],
                )
        b_sb = consts.tile([128, 3], F32, tag="b")
        nc.sync.dma_start(out=b_sb[:], in_=b_d[:, :])
        wo_sb = consts.tile([128, D_MODEL], BF16, tag="wo")
        nc.sync.dma_start(out=wo_sb[:], in_=wo_d[:, :])
        mask_sb = consts.tile([128, 4 * QT], BF16, tag="mask")
        for p in range(4):
            nc.sync.dma_start(out=mask_sb[:, p * QT:(p + 1) * QT], in_=mk_d[p, :, :])
        id_sb = consts.tile([128, 128], BF16, tag="id")
        nc.sync.dma_start(out=id_sb[:], in_=id_d[:, :])

        # ---- x input, chunked per (contraction chunk, tok tile) ----
        xT_sb = []
        for j in range(CCH):
            t = big.tile([128, T], BF16, tag=f"xT{j}")
            xT_sb.append(t)
        for tt in range(NTOKT):
            for j in range(CCH):
                nc.sync.dma_start(
                    out=xT_sb[j][:, tt * TOKT:(tt + 1) * TOKT],
                    in_=xT_d[j * 128:(j + 1) * 128, tt * TOKT:(tt + 1) * TOKT],
                )

        # ---- projections ----
        # Q8/K8: [128, 2T] (second T = zeros for the DoubleRow k-subtile pad)
        qw = 2 * T if SCORES_FP8 else T
        Q8 = big.tile([128, qw], qk_dt, tag="Q8")
        K8 = big.tile([128, qw], qk_dt, tag="K8")
        if SCORES_FP8:
            nc.vector.memset(Q8[:, T:2 * T], 0.0)
            nc.vector.memset(K8[:, T:2 * T], 0.0)
        VT_sb = big.tile([128, T], BF16, tag="VT")
        dests = [Q8, K8, VT_sb]
        for tt in range(NTOKT):
            for s in range(3):
                pp = sp.tile([128, TOKT], F32, tag="sc")
                for j in range(CCH):
                    base = (s * CCH + j) * 128
                    nc.tensor.matmul(
                        pp[:], w_sb[:, base:base + 128],
                        xT_sb[j][:, tt * TOKT:(tt + 1) * TOKT],
                        start=(j == 0), stop=(j == CCH - 1),
                    )
                nc.vector.tensor_scalar_add(
                    dests[s][:, tt * TOKT:(tt + 1) * TOKT],
                    pp[:], b_sb[:, s:s + 1],
                )

        # ---- V2 per 128-token key chunk, stride 256 cols:
        #   [0:64]=V_A  [64:192]=ones  [192:256]=V_B
        # lhsT A = cols 0:128   -> psum rows 0:64 AV_A, 64:128 denom_A (x64)
        # lhsT B = cols 128:256 -> psum rows 0:64 denom_B (x64), 64:128 AV_B
        V_sb = big.tile([128, (T // 128) * VST], BF16, tag="V")
        v3 = V_sb[:].rearrange("p (t c) -> p t c", c=VST)
        nc.vector.memset(v3[:, :, 64:192], 1.0)
        for tt4 in range(T // 128):
            tp = sp.tile([128, 128], BF16, tag="sc")
            nc.tensor.transpose(tp[:], VT_sb[:, tt4 * 128:(tt4 + 1) * 128], id_sb[:])
            # single strided copy: tp cols 0:64 -> V2 cols 0:64 (V_A),
            # tp cols 64:128 -> V2 cols 192:256 (V_B)
            src = tp[:]
            dst = V_sb[:, tt4 * VST:(tt4 + 1) * VST]
            nc.vector.tensor_copy(
                bass.AP(tensor=dst.tensor, offset=dst.offset,
                        ap=[dst.ap[0], [192, 2], [1, 64]]),
                bass.AP(tensor=src.tensor, offset=src.offset,
                        ap=[src.ap[0], [64, 2], [1, 64]]),
            )

        if SCORES_FP8:
            q3 = Q8[:].rearrange("p (s n) -> p s n", s=2)
            k3 = K8[:].rearrange("p (s n) -> p s n", s=2)

        # ---- attention + out-projection ----
        for qi in range(NQT):
            qs = qi * QT
            # av bank 0 (cols 0:QT): slot A; bank 1 (cols QT:2QT): slot B
            av = avp.tile([128, 2 * QT], F32, tag="av")
            nsteps = 4 * (qi + 1)
            jobs = [(kc, h) for kc in range(nsteps) for h in (0, 1)]
            for g in range(0, len(jobs), GRP):
                grp = jobs[g:g + GRP]
                width = len(grp) * QT
                sc = sp.tile([128, GRP * QT], F32, tag="sc")
                for ji, (kc, h) in enumerate(grp):
                    if SCORES_FP8:
                        nc.tensor.matmul(
                            sc[:, ji * QT:(ji + 1) * QT],
                            k3[64 * h:64 * h + 64, :, kc * KC:(kc + 1) * KC],
                            q3[64 * h:64 * h + 64, :, qs:qs + QT],
                            start=True, stop=True, perf_mode=PM.DoubleRow,
                        )
                    else:
                        nc.tensor.matmul(
                            sc[:, ji * QT:(ji + 1) * QT],
                            K8[64 * h:64 * h + 64, kc * KC:(kc + 1) * KC],
                            Q8[64 * h:64 * h + 64, qs:qs + QT],
                            start=True, stop=True,
                        )
                pt = ptp.tile([128, GRP * QT], BF16, tag="pt")
                nc.scalar.activation(
                    pt[:, :width], sc[:, :width], AF.Exp,
                    scale=1.0 / math.sqrt(HEAD_DIM),
                )
                for ji, (kc, h) in enumerate(grp):
                    ptj = pt[:, ji * QT:(ji + 1) * QT]
                    if kc >= 4 * qi:  # diagonal straddle
                        pat = kc - 4 * qi
                        m = mask_sb[:, pat * QT:(pat + 1) * QT]
                        nc.vector.tensor_mul(ptj, ptj, m)
                    st = kc == 0
                    sp_ = kc == nsteps - 1
                    nc.tensor.matmul(
                        av[:, h * QT:(h + 1) * QT],
                        V_sb[:, kc * VST + 128 * h:kc * VST + 128 * h + 128],
                        ptj, start=st, stop=sp_,
                    )
            # normalize: reciprocal of the 64x-replicated denominator blocks
            r = rp.tile([128, QT], F32, tag="r")
            nc.vector.reciprocal(r[64:128, :], av[64:128, 0:QT])
            nc.vector.reciprocal(r[0:64, :], av[0:64, QT:2 * QT])
            # partition-broadcast via DRAM bounce (stride-0 partition reads
            # are only legal from DRAM)
            rd = dramp.tile([1, 2 * QT], F32, tag="rd")
            nc.sync.dma_start(out=rd[0:1, 0:QT], in_=r[64:65, 0:QT])
            nc.sync.dma_start(out=rd[0:1, QT:2 * QT], in_=r[0:1, 0:QT])
            rbc = rp.tile([128, QT], F32, tag="rbc")
            rdA = rd[0:1, 0:QT]
            rdB = rd[0:1, QT:2 * QT]
            nc.gpsimd.dma_start(
                out=rbc[0:64, :],
                in_=bass.AP(tensor=rdA.tensor, offset=rdA.offset,
                            ap=[[0, 64]] + list(rdA.ap[1:])))
            nc.gpsimd.dma_start(
                out=rbc[64:128, :],
                in_=bass.AP(tensor=rdB.tensor, offset=rdB.offset,
                            ap=[[0, 64]] + list(rdB.ap[1:])))
            # fused normalize + PSUM->SBUF copy
            hv = hvp.tile([128, QT], BF16, tag="hv")
            nc.vector.tensor_mul(hv[0:64, :], av[0:64, 0:QT], rbc[0:64, :])
            nc.vector.tensor_mul(hv[64:128, :], av[64:128, QT:2 * QT], rbc[64:128, :])
            # merged out projection for this query tile: outT[dout, q]
            for dch in range(CCH):
                op = sp.tile([128, QT], F32, tag="sc")
                nc.tensor.matmul(
                    op[:], wo_sb[:, dch * 128:(dch + 1) * 128], hv[:],
                    start=True, stop=True,
                )
                ot = osb.tile([128, QT], BF16, tag="ot")
                nc.vector.tensor_copy(ot[:], op[:])
                nc.sync.dma_start(
                    out=outT_d[dch * 128:(dch + 1) * 128, qs:qs + QT], in_=ot[:],
                )
    nc.finalize()
    return nc


def _host_inputs(x, wq, bq, wk, bk, wv, bv, wo):
    """Per-core input maps. Slot A of core c = head c; slot B = head 8+c
    (cores 0-3) or a dummy zero head (cores 4-7)."""
    bf16 = ml_dtypes.bfloat16
    xT = np.ascontiguousarray(x[0].T).astype(bf16)
    masks = np.zeros((4, 128, QT), np.float32)
    dk = np.arange(128)[:, None]
    dq = np.arange(QT)[None, :]
    for p in range(4):
        masks[p] = (dk + 128 * p <= dq)
    masks = masks.astype(bf16)
    ident = np.eye(128, dtype=np.float32).astype(bf16)

    in_maps = []
    for c in range(N_CORES):
        hA = c
        hB = 8 + c if c < 4 else None
        w = np.zeros((3, D_MODEL, 128), np.float32)
        b = np.zeros((128, 3), np.float32)
        wo2 = np.zeros((128, D_MODEL), np.float32)
        for s, (W, B) in enumerate(((wq, bq), (wk, bk), (wv, bv))):
            w[s, :, 0:64] = W[hA]
            b[0:64, s] = B[hA]
            if hB is not None:
                w[s, :, 64:128] = W[hB]
                b[64:128, s] = B[hB]
        wo2[0:64, :] = wo[hA * 64:(hA + 1) * 64, :]
        if hB is not None:
            wo2[64:128, :] = wo[hB * 64:(hB + 1) * 64, :]
        in_maps.append({
            "xT": xT,
            "wqkv": w.astype(bf16),
            "bqkv": b.astype(np.float32),
            "wo2": wo2.astype(bf16),
            "masks": masks,
            "ident": ident,
        })
    return in_maps


def kernel(_trace=False, _tmpdir=None, **inputs):
    x = np.asarray(inputs["x"], np.float32)
    args = (x,
            np.asarray(inputs["wq"], np.float32), np.asarray(inputs["bq"], np.float32),
            np.asarray(inputs["wk"], np.float32), np.asarray(inputs["bk"], np.float32),
            np.asarray(inputs["wv"], np.float32), np.asarray(inputs["bv"], np.float32),
            np.asarray(inputs["wo"], np.float32))
    bo = np.asarray(inputs["bo"], np.float32)

    if "nc" not in _PROGRAM_CACHE:
        _PROGRAM_CACHE["nc"] = build_program()
    nc = _PROGRAM_CACHE["nc"]

    in_maps = _host_inputs(*args)
    res = run_bass_kernel_spmd(
        nc, in_maps, list(range(N_CORES)), trace=_trace, tmpdir=_tmpdir,
    )
    acc = np.zeros((D_MODEL, T), np.float32)
    for c in range(N_CORES):
        acc += res.results[c]["outT"].astype(np.float32)
    out = acc.T + bo[None, :]
    if _trace:
        return out[None].astype(np.float32), res
    return out[None].astype(np.float32)
